# revision 1
# baseline (speedup 1.0000x reference)
"""Trainium2 Bass kernel for nn_Cell_A (capsule cell: conv1d -> squash ->
strided conv2d -> 3-iter dynamic routing).

Sharding: pure data parallel, batch B=8 across 8 NeuronCores. Each core runs
the same NEFF on its own batch element; weights replicated.

Per-core layout: l-major (sequence position on partitions, 8 chunks of 128).
 - conv1/conv2/s0 and all routing reductions run on the PE (PSUM-accumulated
   "copy matmuls" with an identity stationary perform free-dim reductions).
 - elementwise multiplies on DVE; V and M tiles bf16 in (cp, a, csa) order so
   both tensor_tensor operands keep unit inner stride (2x mode).
 - transcendentals on ACT using only the exp/ln table set; the squash factor
   is sq*exp(-(ln(1+sq)+0.5*ln(sq+eps))) to avoid table-set switches.
"""

import os
import sys

import numpy as np

sys.path.insert(0, "/opt/trn_rl_repo")

K, L = 64, 1024
CP, APd, G2 = 32, 8, 9
CSA, ASA, G3 = 16, 16, 3
NCH = CP * APd          # 256 conv1 out channels
NO = CSA * ASA          # 256 conv2 out channels
EPS = 1e-8
NCORES = 8
NCHUNK = 8              # L / 128
PL = 128                # l per chunk

_CACHE = {}
_B2_IS_ZERO = [True]


def _build_nc():
    import concourse.bacc as bacc
    import concourse.mybir as mybir
    import concourse.tile as tile
    from concourse.mybir import ActivationFunctionType as AF, AluOpType as OP

    f32 = mybir.dt.float32
    bf16 = mybir.dt.bfloat16

    # Pin all ACT activations to the one table set containing Exp+Ln+Copy so
    # the table-load pass emits a single hoisted load (no per-call reloads).
    from concourse.hw_specs import get_activation_tables as _gat
    _keep = "natural_log_exp_and_others"
    _used = {AF.Exp, AF.Ln, AF.Copy, AF.Identity, AF.Square}

    def _gat_one(arch):
        tabs = _gat(arch)
        assert _used <= tabs[_keep]
        return {n: (f if n == _keep else (f - _used)) for n, f in tabs.items()}

    bacc.get_activation_tables = _gat_one

    nc = bacc.Bacc("TRN2", target_bir_lowering=False, debug=False)

    x_d = nc.dram_tensor("x", [K, L], f32, kind="ExternalInput")
    w1r_d = nc.dram_tensor("w1r", [K + 1, G2, NCH], f32, kind="ExternalInput")
    w2s_d = nc.dram_tensor("w2s", [128, 4, G3, NO], bf16, kind="ExternalInput")
    wfold_d = nc.dram_tensor("wfold", [128, 2, G3, NO], bf16, kind="ExternalInput")
    b2f_d = nc.dram_tensor("b2f", [NO], f32, kind="ExternalInput")  # 2*b2 perm
    b2p_d = nc.dram_tensor("b2p", [NO], f32, kind="ExternalInput")  # b2 perm
    iden_d = nc.dram_tensor("iden", [128, 128], f32, kind="ExternalInput")
    idbf_d = nc.dram_tensor("idbf", [128, 128], bf16, kind="ExternalInput")
    out_d = nc.dram_tensor("out", [L * CSA, ASA], f32, kind="ExternalOutput")

    out_v = out_d.ap().rearrange("(l s) a -> l s a", s=CSA)

    with tile.TileContext(nc) as tc:
        import contextlib
        ctx = contextlib.ExitStack()
        with ctx:
            singles = ctx.enter_context(tc.tile_pool(name="singles", bufs=1))
            ysqt_p = ctx.enter_context(tc.tile_pool(name="ysqt", bufs=1))
            vpool = ctx.enter_context(tc.tile_pool(name="vpool", bufs=2))
            mpool = ctx.enter_context(tc.tile_pool(name="mpool", bufs=2))
            sm = ctx.enter_context(tc.tile_pool(name="sm", bufs=4))
            smb = ctx.enter_context(tc.tile_pool(name="smb", bufs=2))
            vout = ctx.enter_context(tc.tile_pool(name="vout", bufs=4))
            ps_a = ctx.enter_context(tc.tile_pool(name="ps_a", bufs=3, space="PSUM"))
            ps_s0 = ctx.enter_context(tc.tile_pool(name="ps_s0", bufs=1, space="PSUM"))
            ps_v = ctx.enter_context(tc.tile_pool(name="ps_v", bufs=2, space="PSUM"))
            ps_r = ctx.enter_context(tc.tile_pool(name="ps_r", bufs=2, space="PSUM"))

            # ---- constant / weight loads ----
            xpad = singles.tile([128, L + 8], f32)
            nc.vector.memset(xpad[0:K, 0:4], 0.0)
            nc.vector.memset(xpad[0:K, L + 4:L + 8], 0.0)
            nc.vector.memset(xpad[K:K + 1, :], 1.0)
            nc.sync.dma_start(out=xpad[0:K, 4:4 + L], in_=x_d.ap())

            w1r = singles.tile([128, G2, NCH], f32)
            nc.sync.dma_start(out=w1r[0:K + 1], in_=w1r_d.ap())
            w2s = singles.tile([128, 4, G3, NO], bf16)
            nc.sync.dma_start(out=w2s, in_=w2s_d.ap())
            wfold = singles.tile([128, 2, G3, NO], bf16)
            nc.sync.dma_start(out=wfold, in_=wfold_d.ap())
            iden = singles.tile([128, 128], f32)
            nc.sync.dma_start(out=iden, in_=iden_d.ap())
            idbf = singles.tile([128, 128], bf16)
            nc.sync.dma_start(out=idbf, in_=idbf_d.ap())
            b2frep = singles.tile([128, NO], f32)
            nc.sync.dma_start(
                out=b2frep, in_=b2f_d.ap().unsqueeze(0).broadcast_to([128, NO]))
            b2prep = singles.tile([128, NO], f32)
            nc.sync.dma_start(
                out=b2prep, in_=b2p_d.ap().unsqueeze(0).broadcast_to([128, NO]))
            cst0 = singles.tile([128, 1], f32)
            nc.vector.memset(cst0, 0.0)
            cst1 = singles.tile([128, 1], f32)
            nc.vector.memset(cst1, 1.0)
            cstE = singles.tile([128, 1], f32)
            nc.vector.memset(cstE, EPS)

            # y_sqT: squashed conv1 output, channel-major, l-padded by 1
            ysqt = [ysqt_p.tile([128, L + 2], bf16, tag=f"ysqt{h}",
                                name=f"ysqt{h}") for h in range(2)]
            # rows 96-127 of each half re-based at partitions 0-31 / 32-63
            # (matmul base_partition must be in {0, 32, 64})
            ysqx = ysqt_p.tile([64, L + 2], bf16, tag="ysqx", name="ysqx")
            for h in range(2):
                nc.vector.memset(ysqt[h][:, 0:1], 0.0)
                nc.vector.memset(ysqt[h][:, L + 1:L + 2], 0.0)
            nc.vector.memset(ysqx[:, 0:1], 0.0)
            nc.vector.memset(ysqx[:, L + 1:L + 2], 0.0)

            def squash_factor(sq, pool, n):
                """f = sq/((1+sq)*sqrt(sq+eps)) = sq*exp(-(ln(1+sq)+.5ln(sq+eps)))
                sq: [128, n] fp32. Returns f [128, n]."""
                l1 = pool.tile([128, n], f32, tag="sqf_l1", name="sqf_l1")
                nc.scalar.activation(out=l1, in_=sq, func=AF.Ln,
                                     bias=cst1[:, 0:1], scale=1.0)
                l2 = pool.tile([128, n], f32, tag="sqf_l2", name="sqf_l2")
                nc.scalar.activation(out=l2, in_=sq, func=AF.Ln,
                                     bias=cstE[:, 0:1], scale=1.0)
                t = pool.tile([128, n], f32, tag="sqf_t", name="sqf_t")
                nc.vector.scalar_tensor_tensor(
                    out=t, in0=l2, scalar=0.5, in1=l1, op0=OP.mult, op1=OP.add)
                e = pool.tile([128, n], f32, tag="sqf_e", name="sqf_e")
                nc.scalar.activation(out=e, in_=t, func=AF.Exp,
                                     bias=cst0[:, 0:1], scale=-1.0)
                f = pool.tile([128, n], f32, tag="sqf_f", name="sqf_f")
                nc.vector.tensor_mul(f, sq, e)
                return f

            # -------------- stage A: conv1 + squash + transpose --------------
            for c in range(NCHUNK):
                yps = ps_a.tile([128, NCH], f32, tag="aps", name="yps")
                for t in range(G2):
                    nc.tensor.matmul(
                        yps, lhsT=xpad[0:K + 1, c * PL + t: c * PL + t + PL],
                        rhs=w1r[0:K + 1, t, :],
                        start=(t == 0), stop=(t == G2 - 1))
                ycp = sm.tile([128, NCH], f32, tag="ycp", name="ycp")
                nc.scalar.copy(out=ycp, in_=yps)
                y2 = sm.tile([128, NCH], f32, tag="y2", name="y2")
                nc.vector.tensor_mul(y2, ycp, ycp)
                sq = sm.tile([128, CP], f32, tag="sq1", name="sq1")
                nc.vector.tensor_reduce(
                    out=sq, in_=y2.rearrange("p (c a) -> p c a", a=APd),
                    axis=mybir.AxisListType.X, op=OP.add)
                f = squash_factor(sq, sm, CP)
                ysq = sm.tile([128, NCH], f32, tag="ysq", name="ysq")
                nc.vector.tensor_tensor(
                    out=ysq.rearrange("p (c a) -> p c a", a=APd),
                    in0=ycp.rearrange("p (c a) -> p c a", a=APd),
                    in1=f.unsqueeze(2).broadcast_to([128, CP, APd]),
                    op=OP.mult)
                for h in range(2):
                    tps = ps_a.tile([128, 128], f32, tag="aps", name="tps")
                    nc.tensor.transpose(
                        tps, in_=ysq[:, h * 128:(h + 1) * 128], identity=iden)
                    nc.scalar.copy(
                        out=ysqt[h][:, 1 + c * PL: 1 + (c + 1) * PL], in_=tps)
                    nc.scalar.copy(
                        out=ysqx[h * 32:(h + 1) * 32,
                                 1 + c * PL: 1 + (c + 1) * PL],
                        in_=tps[96:128])

            # ---------- stage B/C per chunk: conv2 + routing ----------
            # column (o) order of w2s/wfold is host-permuted to (a, csa)
            for c in range(NCHUNK):
                # s0 = (1/16) sum_cp V via folded dense matmul; cols (a,csa)
                s0ps = ps_s0.tile([128, NO], f32, tag="s0ps", name="s0ps")
                first = True
                for h in range(2):
                    for dh in range(G3):
                        nc.tensor.matmul(
                            s0ps, lhsT=ysqt[h][:, c * PL + dh: c * PL + dh + PL],
                            rhs=wfold[:, h, dh, :],
                            start=first, stop=(h == 1 and dh == G3 - 1))
                        first = False

                # conv2 -> V chunk, bf16, layout [128, cp, a, csa]
                vt = vpool.tile([128, CP, ASA, CSA], bf16, tag="vt", name="vt")
                for q in range(CP // 2):        # 2 cp per psum group (1 bank)
                    vps = ps_v.tile([128, 2 * NO], f32, tag="vps", name="vps")
                    for j in range(2):
                        cp = q * 2 + j
                        h = cp // 16
                        brow = 32 * ((cp % 16) // 4)
                        r = cp % 4
                        if brow < 96:
                            src_t, b = ysqt[h], brow
                        else:
                            src_t, b = ysqx, 32 * h
                        for dh in range(G3):
                            nc.tensor.matmul(
                                vps[:, j * NO:(j + 1) * NO],
                                lhsT=src_t[b:b + 32,
                                           c * PL + dh: c * PL + dh + PL],
                                rhs=w2s[b:b + 32, r, dh, :],
                                start=(j == 0 and dh == 0),
                                stop=(j == 1 and dh == G3 - 1))
                    # evacuate 4 cp at once (cast to bf16); alternate ACT/DVE
                    dst = vt[:, q * 2:(q + 1) * 2, :, :].rearrange(
                        "p c a s -> p (c a s)")
                    if _B2_IS_ZERO[0]:
                        if q % 2 == 0:
                            nc.scalar.activation(out=dst, in_=vps, func=AF.Copy,
                                                 bias=0.0, scale=1.0)
                        else:
                            nc.vector.tensor_copy(dst, vps)
                    else:
                        nc.vector.scalar_tensor_tensor(
                            out=dst, in0=vps, scalar=1.0,
                            in1=b2prep.unsqueeze(1).broadcast_to([128, 2, NO])
                            .rearrange("p c s -> p (c s)"),
                            op0=OP.mult, op1=OP.add)

                # v0 = squash(s0 + 2*b2)   (s0 cols are (a, csa))
                s0b = smb.tile([128, NO], f32, tag="s0b", name="s0b")
                nc.vector.tensor_add(s0b, s0ps, b2frep)
                s_in = s0b.rearrange("p (a s) -> p a s", s=CSA)
                bt_prev = None
                for r in (1, 2):
                    # squash(s_in) -> vprev [128, a, csa] bf16
                    if r > 1:
                        s_sb = smb.tile([128, NO], f32, tag="s_sb", name="s_sb")
                        nc.scalar.copy(out=s_sb, in_=s_in)
                        s_in = s_sb.rearrange("p (a s) -> p a s", s=CSA)
                    s2 = smb.tile([128, NO], f32, tag="s2", name="s2")
                    nc.vector.tensor_mul(
                        s2.rearrange("p (a s) -> p a s", s=CSA), s_in, s_in)
                    sqs = smb.tile([128, CSA], f32, tag="sqs", name="sqs")
                    nc.vector.tensor_reduce(
                        out=sqs,
                        in_=s2.rearrange("p (a s) -> p a s", s=CSA)
                        .transpose([0, 2, 1]),
                        axis=mybir.AxisListType.X, op=OP.add)
                    fs = squash_factor(sqs, smb, CSA)
                    vprev = vout.tile([128, ASA, CSA], bf16, tag="vprev",
                                      name="vprev")
                    nc.vector.tensor_tensor(
                        out=vprev, in0=s_in,
                        in1=fs.unsqueeze(1).broadcast_to([128, ASA, CSA]),
                        op=OP.mult)

                    # P-op: M = V * v_prev (bcast over cp); P = sum_a M
                    mt = mpool.tile([128, CP, ASA, CSA], bf16, tag="mt",
                                    name="mt")
                    nc.vector.tensor_tensor(
                        out=mt, in0=vt,
                        in1=vprev.unsqueeze(1).broadcast_to(
                            [128, CP, ASA, CSA]),
                        op=OP.mult)
                    pps = ps_r.tile([128, CP * CSA], f32, tag="pps", name="pps")
                    for ai in range(ASA):
                        nc.tensor.matmul(
                            pps.rearrange("p (c s) -> p c s", s=CSA),
                            lhsT=idbf, rhs=mt[:, :, ai, :],
                            start=(ai == 0), stop=(ai == ASA - 1))
                    # b += P ; softmax over csa (no max-sub; logits are small)
                    bt = smb.tile([128, CP * CSA], f32, tag="bt", name="bt")
                    if r == 1:
                        nc.vector.tensor_copy(bt, pps)
                    else:
                        nc.vector.tensor_add(bt, bt_prev, pps)
                    bt_prev = bt
                    et = smb.tile([128, CP * CSA], f32, tag="et", name="et")
                    nc.scalar.activation(out=et, in_=bt, func=AF.Exp,
                                         bias=cst0[:, 0:1], scale=1.0)
                    zt = smb.tile([128, CP], f32, tag="zt", name="zt")
                    nc.vector.tensor_reduce(
                        out=zt, in_=et.rearrange("p (c s) -> p c s", s=CSA),
                        axis=mybir.AxisListType.X, op=OP.add)
                    rz = smb.tile([128, CP], f32, tag="rz", name="rz")
                    nc.vector.reciprocal(rz, zt)
                    ct = smb.tile([128, CP, CSA], bf16, tag="ct", name="ct")
                    nc.vector.tensor_tensor(
                        out=ct, in0=et.rearrange("p (c s) -> p c s", s=CSA),
                        in1=rz.unsqueeze(2).broadcast_to([128, CP, CSA]),
                        op=OP.mult)

                    # s-op: M2 = V * c (bcast over a); s = sum_cp M2
                    mt2 = mpool.tile([128, CP, ASA, CSA], bf16, tag="mt",
                                     name="mt2")
                    nc.vector.tensor_tensor(
                        out=mt2, in0=vt,
                        in1=ct.unsqueeze(2).broadcast_to([128, CP, ASA, CSA]),
                        op=OP.mult)
                    sps = ps_r.tile([128, NO], f32, tag="pps", name="sps")
                    for cpi in range(CP):
                        nc.tensor.matmul(
                            sps, lhsT=idbf,
                            rhs=mt2[:, cpi, :, :].rearrange("p a s -> p (a s)"),
                            start=(cpi == 0), stop=(cpi == CP - 1))
                    s_in = sps.rearrange("p (a s) -> p a s", s=CSA)

                # final squash of last s -> v2, DMA out
                s_sbf = smb.tile([128, NO], f32, tag="s_sb", name="s_sbf")
                nc.scalar.copy(out=s_sbf, in_=s_in)
                s_in = s_sbf.rearrange("p (a s) -> p a s", s=CSA)
                s2 = smb.tile([128, NO], f32, tag="s2", name="s2f")
                nc.vector.tensor_mul(
                    s2.rearrange("p (a s) -> p a s", s=CSA), s_in, s_in)
                sqs = smb.tile([128, CSA], f32, tag="sqs", name="sqsf")
                nc.vector.tensor_reduce(
                    out=sqs,
                    in_=s2.rearrange("p (a s) -> p a s", s=CSA)
                    .transpose([0, 2, 1]),
                    axis=mybir.AxisListType.X, op=OP.add)
                fs = squash_factor(sqs, smb, CSA)
                v2 = vout.tile([128, CSA, ASA], f32, tag="v2", name="v2")
                nc.vector.tensor_tensor(
                    out=v2.transpose([0, 2, 1]), in0=s_in,
                    in1=fs.unsqueeze(1).broadcast_to([128, ASA, CSA]),
                    op=OP.mult)
                nc.sync.dma_start(
                    out=out_v[c * PL:(c + 1) * PL], in_=v2)
    nc.compile()
    return nc


def _prep_weights(w1, b1, w2, b2):
    w1 = np.asarray(w1, np.float32)
    w2 = np.asarray(w2, np.float32)
    b1 = np.asarray(b1, np.float32)
    b2 = np.asarray(b2, np.float32)
    # o-permutation: new column order (a, csa): perm[a*CSA+csa] = csa*ASA+a
    a_i, s_i = np.meshgrid(np.arange(ASA), np.arange(CSA), indexing="ij")
    perm = (s_i * ASA + a_i).reshape(-1)
    w1r = np.zeros((K + 1, G2, NCH), np.float32)
    w1r[0:K] = np.transpose(w1, (1, 2, 0))          # [k, t, o]
    w1r[K, (G2 - 1) // 2, :] = b1                    # bias via ones-row
    w2m = w2[:, 0, :, :]                             # [o, dh, ap]
    w2p = w2m[perm]                                  # permuted o
    w2s = np.zeros((32, 4, G3, NO), np.float32)
    for r in range(4):
        for dh in range(G3):
            w2s[8 * r:8 * r + 8, r, dh, :] = w2p[:, dh, :].T
    w2s = np.concatenate([w2s] * 4, axis=0)          # all 4 row-groups
    wfold = np.zeros((128, 2, G3, NO), np.float32)
    co = np.arange(256)
    for dh in range(G3):
        wf = (w2p[:, dh, :].T[co % 8, :] / float(CSA))   # [co, o']
        wfold[:, 0, dh, :] = wf[0:128]
        wfold[:, 1, dh, :] = wf[128:256]
    import ml_dtypes
    return {
        "w1r": w1r,
        "w2s": w2s.astype(ml_dtypes.bfloat16),
        "wfold": wfold.astype(ml_dtypes.bfloat16),
        "b2p": np.ascontiguousarray(b2[perm]),
        "b2f": np.ascontiguousarray(2.0 * b2[perm]).astype(np.float32),
        "iden": np.eye(128, dtype=np.float32),
    }


def kernel(x, w1, b1, w2, b2):
    import ml_dtypes
    from concourse.bass_utils import run_bass_kernel_spmd

    x = np.asarray(x, np.float32)
    _B2_IS_ZERO[0] = bool(np.all(np.asarray(b2) == 0.0))
    if "nc" not in _CACHE:
        _CACHE["nc"] = _build_nc()
    nc = _CACHE["nc"]
    wmaps = _prep_weights(w1, b1, w2, b2)
    wmaps["idbf"] = np.eye(128).astype(ml_dtypes.bfloat16)
    in_maps = []
    for b in range(NCORES):
        m = {"x": np.ascontiguousarray(x[b])}
        m.update(wmaps)
        in_maps.append(m)
    trace = bool(int(os.environ.get("KERNEL_TRACE", "0")))
    res = run_bass_kernel_spmd(
        nc, in_maps, core_ids=list(range(NCORES)), trace=trace)
    if trace:
        _CACHE["last_exec_time_ns"] = res.exec_time_ns
        _CACHE["last_trace"] = res.instructions_and_trace
    out = np.stack([r["out"] for r in res.results])   # [8, 16384, 16]
    return out.astype(np.float32)



# revision 15
# speedup vs baseline: 1.0546x; 1.0546x over previous
"""Trainium2 Bass kernel for nn_Cell_A (capsule cell: conv1d -> squash ->
strided conv2d -> 3-iter dynamic routing).

Sharding: pure data parallel, batch B=8 across 8 NeuronCores. Each core runs
the same NEFF on its own batch element; weights replicated.

Per-core layout: l-major (sequence position on partitions, 8 chunks of 128).
 - conv1 in bf16 (PE), squash squares on ACT directly from PSUM.
 - conv2 runs 4-way row-tiled: quads (same r=cp%4, row blocks 0/32/64/96)
   execute concurrently in distinct 32x32 PE row groups.
 - routing multiplies on DVE bf16 (2x mode); reductions are PSUM-accumulated
   identity matmuls on the PE; the r2 logit update accumulates onto the r1
   PSUM bank (b2 = b1 + P(v1)) so no logit copy/add is needed.
 - transcendentals on ACT using only the exp/ln table set; 1/z = exp(-ln z);
   squash factor is sq*exp(-(ln(1+sq)+0.5*ln(sq+eps))).
 - small elementwise helpers (softmax c-scale, squash glue) on GPSIMD.
"""

import os
import sys

import numpy as np

sys.path.insert(0, "/opt/trn_rl_repo")

K, L = 64, 1024
CP, APd, G2 = 32, 8, 9
CSA, ASA, G3 = 16, 16, 3
NCH = CP * APd          # 256 conv1 out channels
NO = CSA * ASA          # 256 conv2 out channels
EPS = 1e-8
NCORES = 8
NCHUNK = 8              # L / 128
PL = 128                # l per chunk

_CACHE = {}
_B2_IS_ZERO = [True]


def _build_nc():
    import concourse.bacc as bacc
    import concourse.mybir as mybir
    import concourse.tile as tile
    from concourse.mybir import ActivationFunctionType as AF, AluOpType as OP

    f32 = mybir.dt.float32
    bf16 = mybir.dt.bfloat16

    # Pin all ACT activations to the one table set containing Exp+Ln+Copy so
    # the table-load pass emits a single hoisted load (no per-call reloads).
    from concourse.hw_specs import get_activation_tables as _gat
    _keep = "natural_log_exp_and_others"
    _used = {AF.Exp, AF.Ln, AF.Copy, AF.Identity, AF.Square}

    def _gat_one(arch):
        tabs = _gat(arch)
        assert _used <= tabs[_keep]
        return {n: (f if n == _keep else (f - _used)) for n, f in tabs.items()}

    bacc.get_activation_tables = _gat_one

    nc = bacc.Bacc("TRN2", target_bir_lowering=False, debug=False)

    x_d = nc.dram_tensor("x", [K, L], bf16, kind="ExternalInput")
    w1r_d = nc.dram_tensor("w1r", [K + 1, G2, NCH], bf16, kind="ExternalInput")
    w2bd_d = nc.dram_tensor("w2bd", [128, 4 * NO], bf16, kind="ExternalInput")
    w2s0_d = nc.dram_tensor("w2s0", [128, NO], bf16, kind="ExternalInput")
    b2f_d = nc.dram_tensor("b2f", [NO], f32, kind="ExternalInput")  # 2*b2 perm
    b2p_d = nc.dram_tensor("b2p", [NO], f32, kind="ExternalInput")  # b2 perm
    idbf_d = nc.dram_tensor("idbf", [128, 128], bf16, kind="ExternalInput")
    out_d = nc.dram_tensor("out", [L * CSA, ASA], f32, kind="ExternalOutput")
    DBG = bool(int(os.environ.get("KERNEL_DEBUG_V", "0")))
    if DBG:
        vdbg_d = nc.dram_tensor("vdbg", [NCHUNK, 128, CP * NO], bf16,
                                kind="ExternalOutput")
        ydbg_d = nc.dram_tensor("ydbg", [128, 8, L], bf16,
                                kind="ExternalOutput")
        sdbg_d = nc.dram_tensor("sdbg", [NCHUNK, 128, NO], f32,
                                kind="ExternalOutput")

    out_v = out_d.ap().rearrange("(l s) a -> l s a", s=CSA)

    with tile.TileContext(nc) as tc:
        import contextlib
        ctx = contextlib.ExitStack()
        with ctx:
            singles = ctx.enter_context(tc.tile_pool(name="singles", bufs=1))
            ysqt_p = ctx.enter_context(tc.tile_pool(name="ysqt", bufs=1))
            vpool = ctx.enter_context(tc.tile_pool(name="vpool", bufs=2))
            mpool = ctx.enter_context(tc.tile_pool(name="mpool", bufs=2))
            sm = ctx.enter_context(tc.tile_pool(name="sm", bufs=4))
            smb = ctx.enter_context(tc.tile_pool(name="smb", bufs=4))
            vout = ctx.enter_context(tc.tile_pool(name="vout", bufs=4))
            ps_a = ctx.enter_context(tc.tile_pool(name="ps_a", bufs=2, space="PSUM"))
            ps_s = ctx.enter_context(tc.tile_pool(name="ps_s", bufs=2, space="PSUM"))
            ps_v = ctx.enter_context(tc.tile_pool(name="ps_v", bufs=1, space="PSUM"))
            ps_p = ctx.enter_context(tc.tile_pool(name="ps_p", bufs=2, space="PSUM"))

            # ---- constant / weight loads ----
            xpad = singles.tile([128, L + 8], bf16)
            nc.vector.memset(xpad[0:K, 0:4], 0.0)
            nc.vector.memset(xpad[0:K, L + 4:L + 8], 0.0)
            nc.vector.memset(xpad[K:K + 1, :], 1.0)
            nc.sync.dma_start(out=xpad[0:K, 4:4 + L], in_=x_d.ap())

            w1r = singles.tile([128, G2, NCH], bf16)
            nc.sync.dma_start(out=w1r[0:K + 1], in_=w1r_d.ap())
            w2bd = singles.tile([128, 4 * NO], bf16)
            nc.sync.dma_start(out=w2bd, in_=w2bd_d.ap())
            w2s0 = singles.tile([128, NO], bf16)
            nc.sync.dma_start(out=w2s0, in_=w2s0_d.ap())
            idbf = singles.tile([128, 128], bf16)
            nc.sync.dma_start(out=idbf, in_=idbf_d.ap())
            b2frep = singles.tile([128, NO], f32)
            nc.sync.dma_start(
                out=b2frep, in_=b2f_d.ap().unsqueeze(0).broadcast_to([128, NO]))
            b2prep = singles.tile([128, NO], f32)
            nc.sync.dma_start(
                out=b2prep, in_=b2p_d.ap().unsqueeze(0).broadcast_to([128, NO]))
            cst0 = singles.tile([128, 1], f32)
            nc.vector.memset(cst0, 0.0)
            cst1 = singles.tile([128, 1], f32)
            nc.vector.memset(cst1, 1.0)
            cstE = singles.tile([128, 1], f32)
            nc.vector.memset(cstE, EPS)

            # y_sqT: squashed conv1 output, channel-major (DMA staging)
            ysqt = [ysqt_p.tile([128, L], bf16, tag=f"ysqt{h}",
                                name=f"ysqt{h}") for h in range(2)]
            # yrep: per slot t (cps 4t..4t+3), row 32*dh + 8*g + ap holds
            # ysq[(4t+g, ap), l + dh - 1]; rows 96..127 are zero pad.
            yrep = ysqt_p.tile([128, 8, L], bf16, tag="yrep", name="yrep")
            # zero everything once: pad rows and the dh-shift edge columns
            # stay zero; the per-chunk shifted-copy DMAs fill the data rows.
            nc.gpsimd.memset(yrep, 0.0)

            def squash_factor(sq, pool, n, tg):
                """f = sq/((1+sq)*sqrt(sq+eps)) = sq*exp(-(ln(1+sq)+.5ln(sq+eps)))
                sq: [128, n] fp32. Returns f [128, n]. ACT + GPSIMD only."""
                l1 = pool.tile([128, n], f32, tag=f"{tg}_l1", name="sqf_l1")
                nc.scalar.activation(out=l1, in_=sq, func=AF.Ln,
                                     bias=cst1[:, 0:1], scale=1.0)
                l2 = pool.tile([128, n], f32, tag=f"{tg}_l2", name="sqf_l2")
                nc.scalar.activation(out=l2, in_=sq, func=AF.Ln,
                                     bias=cstE[:, 0:1], scale=1.0)
                t = pool.tile([128, n], f32, tag=f"{tg}_t", name="sqf_t")
                nc.vector.scalar_tensor_tensor(
                    out=t, in0=l2, scalar=0.5, in1=l1, op0=OP.mult, op1=OP.add)
                e = pool.tile([128, n], f32, tag=f"{tg}_e", name="sqf_e")
                nc.scalar.activation(out=e, in_=t, func=AF.Exp,
                                     bias=cst0[:, 0:1], scale=-1.0)
                f = pool.tile([128, n], f32, tag=f"{tg}_f", name="sqf_f")
                nc.vector.tensor_tensor(out=f, in0=sq, in1=e, op=OP.mult)
                return f

            # -------------- stage A: conv1 + squash + transpose --------------
            for c in range(NCHUNK):
                yps = ps_a.tile([128, NCH], f32, tag="aps", name="yps")
                for t in range(G2):
                    nc.tensor.matmul(
                        yps, lhsT=xpad[0:K + 1, c * PL + t: c * PL + t + PL],
                        rhs=w1r[0:K + 1, t, :],
                        start=(t == 0), stop=(t == G2 - 1))
                y2 = sm.tile([128, NCH], f32, tag="y2", name="y2")
                nc.scalar.activation(out=y2, in_=yps, func=AF.Square,
                                     bias=cst0[:, 0:1], scale=1.0)
                sq = sm.tile([128, CP], f32, tag="sq1", name="sq1")
                nc.vector.tensor_reduce(
                    out=sq, in_=y2.rearrange("p (c a) -> p c a", a=APd),
                    axis=mybir.AxisListType.X, op=OP.add)
                f = squash_factor(sq, sm, CP, "sa")
                ysq = sm.tile([128, NCH], bf16, tag="ysq", name="ysq")
                nc.vector.tensor_tensor(
                    out=ysq.rearrange("p (c a) -> p c a", a=APd),
                    in0=yps.rearrange("p (c a) -> p c a", a=APd),
                    in1=f.unsqueeze(2).broadcast_to([128, CP, APd]),
                    op=OP.mult)
                for h in range(2):
                    tps = ps_a.tile([128, 128], bf16, tag="aps", name="tps")
                    nc.tensor.transpose(
                        tps, in_=ysq[:, h * 128:(h + 1) * 128], identity=idbf)
                    nc.scalar.copy(
                        out=ysqt[h][:, c * PL: (c + 1) * PL], in_=tps)
                # replicate into yrep: 3 dh-shifted contiguous 32-row copies
                for dh in range(G3):
                    sh = 1 - dh
                    s_lo = c * PL
                    s_hi = c * PL + PL
                    d_lo, d_hi = s_lo + sh, s_hi + sh
                    if d_lo < 0:
                        s_lo += -d_lo
                        d_lo = 0
                    if d_hi > L:
                        s_hi -= d_hi - L
                        d_hi = L
                    for t in range(8):
                        h = t // 4
                        tl = t - 4 * h
                        nc.sync.dma_start(
                            out=yrep[32 * dh:32 * dh + 32, t, d_lo:d_hi],
                            in_=ysqt[h][32 * tl:32 * tl + 32, s_lo:s_hi])

            # ---------- stage B/C per chunk: conv2 + routing ----------
            # column (o) order of w2s/wfold is host-permuted to (a, csa)
            for c in range(NCHUNK):
                # s0 = (1/16) sum_cp V: yrep slots vs w2s0, PSUM-accumulated
                s0ps = ps_s.tile([128, NO], f32, tag="sps", name="s0ps")
                for t in range(8):
                    nc.tensor.matmul(
                        s0ps, lhsT=yrep[:, t, c * PL:(c + 1) * PL],
                        rhs=w2s0, start=(t == 0), stop=(t == 7))

                # conv2 -> V chunk, bf16, layout [128, cp, a, csa]
                # slot t covers cps 4t..4t+3; block-diagonal w2bd gives two
                # cps per plain 512-col matmul (out cols (g, (a,csa))).
                vt = vpool.tile([128, CP, ASA, CSA], bf16, tag="vt", name="vt")
                for t in range(8):
                    for j in range(2):
                        vq = ps_v.tile([128, 2 * NO], f32, tag="vq", name="vq")
                        nc.tensor.matmul(
                            vq, lhsT=yrep[:, t, c * PL:(c + 1) * PL],
                            rhs=w2bd[:, 2 * j * NO:(2 * j + 2) * NO],
                            start=True, stop=True)
                        dst = vt[:, 4 * t + 2 * j:4 * t + 2 * j + 2, :, :]
                        if _B2_IS_ZERO[0]:
                            nc.scalar.copy(out=dst, in_=vq)
                        else:
                            nc.vector.scalar_tensor_tensor(
                                out=dst.rearrange("p c a s -> p (c a s)"),
                                in0=vq, scalar=1.0,
                                in1=b2prep.unsqueeze(1).broadcast_to(
                                    [128, 2, NO]).rearrange(
                                    "p c s -> p (c s)"),
                                op0=OP.mult, op1=OP.add)

                if DBG:
                    nc.sync.dma_start(
                        out=vdbg_d.ap()[c],
                        in_=vt.rearrange("p c a s -> p (c a s)"))
                    sdbgs = smb.tile([128, NO], f32, tag="sdbg", name="sdbgs")
                    nc.scalar.copy(out=sdbgs, in_=s0ps)
                    nc.sync.dma_start(out=sdbg_d.ap()[c], in_=sdbgs)
                    if c == NCHUNK - 1:
                        nc.sync.dma_start(out=ydbg_d.ap(), in_=yrep)

                # ---- routing ----
                def squash_psum(spsrc, tg, out_tile, out_view):
                    """squash(s) from PSUM s [128, (a s)]; writes out_tile."""
                    s2 = smb.tile([128, NO], f32, tag=f"{tg}_s2", name="s2")
                    nc.scalar.activation(out=s2, in_=spsrc, func=AF.Square,
                                         bias=cst0[:, 0:1], scale=1.0)
                    sqs = smb.tile([128, CSA], f32, tag=f"{tg}_sqs", name="sqs")
                    nc.vector.tensor_reduce(
                        out=sqs,
                        in_=s2.rearrange("p (a s) -> p a s", s=CSA)
                        .transpose([0, 2, 1]),
                        axis=mybir.AxisListType.X, op=OP.add)
                    fs = squash_factor(sqs, smb, CSA, tg)
                    nc.vector.tensor_tensor(
                        out=out_view,
                        in0=spsrc.rearrange("p (a s) -> p a s", s=CSA),
                        in1=fs.unsqueeze(1).broadcast_to([128, ASA, CSA]),
                        op=OP.mult)
                    return out_tile

                if _B2_IS_ZERO[0]:
                    s_src = s0ps
                else:
                    s0b = smb.tile([128, NO], f32, tag="s0b", name="s0b")
                    nc.vector.tensor_add(s0b, s0ps, b2frep)
                    s_src = s0b

                pps = ps_p.tile([128, CP * CSA], f32, tag="pps", name="pps")
                sps_last = None
                for r in (1, 2):
                    vprev = vout.tile([128, ASA, CSA], bf16, tag="vprev",
                                      name="vprev")
                    squash_psum(s_src, f"r{r}", vprev, vprev)

                    # P-op: M = V * v_prev (bcast over cp); P = sum_a M
                    mt = mpool.tile([128, CP, ASA, CSA], bf16, tag="mt",
                                    name="mt")
                    nc.vector.tensor_tensor(
                        out=mt, in0=vt,
                        in1=vprev.unsqueeze(1).broadcast_to(
                            [128, CP, ASA, CSA]),
                        op=OP.mult)
                    # logits accumulate in PSUM across iterations (b = sum P)
                    for ai in range(ASA):
                        nc.tensor.matmul(
                            pps.rearrange("p (c s) -> p c s", s=CSA),
                            lhsT=idbf, rhs=mt[:, :, ai, :],
                            start=(r == 1 and ai == 0), stop=(ai == ASA - 1),
                            skip_group_check=True)
                    # softmax over csa (no max-sub; logits are small)
                    et = smb.tile([128, CP, CSA], bf16, tag="et", name="et")
                    nc.scalar.activation(
                        out=et, in_=pps.rearrange("p (c s) -> p c s", s=CSA),
                        func=AF.Exp, bias=cst0[:, 0:1], scale=1.0)
                    zt = smb.tile([128, CP], f32, tag="zt", name="zt")
                    nc.vector.tensor_reduce(
                        out=zt, in_=et, axis=mybir.AxisListType.X, op=OP.add)
                    lnz = smb.tile([128, CP], f32, tag="lnz", name="lnz")
                    nc.scalar.activation(out=lnz, in_=zt, func=AF.Ln,
                                         bias=cst0[:, 0:1], scale=1.0)
                    rz = smb.tile([128, CP], f32, tag="rz", name="rz")
                    nc.scalar.activation(out=rz, in_=lnz, func=AF.Exp,
                                         bias=cst0[:, 0:1], scale=-1.0)
                    ct = smb.tile([128, CP, CSA], bf16, tag="ct", name="ct")
                    nc.vector.tensor_tensor(
                        out=ct, in0=et,
                        in1=rz.unsqueeze(2).broadcast_to([128, CP, CSA]),
                        op=OP.mult)

                    # s-op: M2 = V * c (bcast over a); s = sum_cp M2
                    mt2 = mpool.tile([128, CP, ASA, CSA], bf16, tag="mt",
                                     name="mt2")
                    nc.vector.tensor_tensor(
                        out=mt2, in0=vt,
                        in1=ct.unsqueeze(2).broadcast_to([128, CP, ASA, CSA]),
                        op=OP.mult)
                    sps = ps_s.tile([128, NO], f32, tag="sps", name="sps")
                    for cpi in range(CP):
                        nc.tensor.matmul(
                            sps, lhsT=idbf,
                            rhs=mt2[:, cpi, :, :].rearrange("p a s -> p (a s)"),
                            start=(cpi == 0), stop=(cpi == CP - 1))
                    s_src = sps
                    sps_last = sps

                # final squash of last s -> v2, DMA out
                v2 = vout.tile([128, CSA, ASA], f32, tag="v2", name="v2")
                squash_psum(sps_last, "rf", v2, v2.transpose([0, 2, 1]))
                nc.sync.dma_start(
                    out=out_v[c * PL:(c + 1) * PL], in_=v2)
    nc.compile()
    return nc


def _prep_weights(w1, b1, w2, b2):
    import ml_dtypes
    w1 = np.asarray(w1, np.float32)
    w2 = np.asarray(w2, np.float32)
    b1 = np.asarray(b1, np.float32)
    b2 = np.asarray(b2, np.float32)
    # o-permutation: new column order (a, csa): perm[a*CSA+csa] = csa*ASA+a
    a_i, s_i = np.meshgrid(np.arange(ASA), np.arange(CSA), indexing="ij")
    perm = (s_i * ASA + a_i).reshape(-1)
    w1r = np.zeros((K + 1, G2, NCH), np.float32)
    w1r[0:K] = np.transpose(w1, (1, 2, 0))          # [k, t, o]
    w1r[K, (G2 - 1) // 2, :] = b1                    # bias via ones-row
    w2m = w2[:, 0, :, :]                             # [o, dh, ap]
    w2p = w2m[perm]                                  # permuted o
    # rows 32*dh + 8*g + ap; cols (g', o') block-diagonal
    w2bd = np.zeros((128, 4 * NO), np.float32)
    w2s0 = np.zeros((128, NO), np.float32)
    for g in range(4):
        for dh in range(G3):
            rows = slice(32 * dh + 8 * g, 32 * dh + 8 * g + 8)
            w2bd[rows, g * NO:(g + 1) * NO] = w2p[:, dh, :].T
            w2s0[rows, :] = w2p[:, dh, :].T / float(CSA)
    return {
        "w1r": w1r.astype(ml_dtypes.bfloat16),
        "w2bd": w2bd.astype(ml_dtypes.bfloat16),
        "w2s0": w2s0.astype(ml_dtypes.bfloat16),
        "b2p": np.ascontiguousarray(b2[perm]),
        "b2f": np.ascontiguousarray(2.0 * b2[perm]).astype(np.float32),
        "idbf": np.eye(128).astype(ml_dtypes.bfloat16),
    }


def kernel(x, w1, b1, w2, b2):
    import ml_dtypes
    from concourse.bass_utils import run_bass_kernel_spmd

    x = np.asarray(x, np.float32)
    _B2_IS_ZERO[0] = bool(np.all(np.asarray(b2) == 0.0))
    if "nc" not in _CACHE:
        _CACHE["nc"] = _build_nc()
    nc = _CACHE["nc"]
    wmaps = _prep_weights(w1, b1, w2, b2)
    in_maps = []
    for b in range(NCORES):
        m = {"x": np.ascontiguousarray(x[b]).astype(ml_dtypes.bfloat16)}
        m.update(wmaps)
        in_maps.append(m)
    trace = bool(int(os.environ.get("KERNEL_TRACE", "0")))
    res = run_bass_kernel_spmd(
        nc, in_maps, core_ids=list(range(NCORES)), trace=trace)
    if trace:
        _CACHE["last_exec_time_ns"] = res.exec_time_ns
        _CACHE["last_trace"] = res.instructions_and_trace
    out = np.stack([r["out"] for r in res.results])   # [8, 16384, 16]
    return out.astype(np.float32)


# revision 16
# speedup vs baseline: 1.2162x; 1.1532x over previous
"""Trainium2 Bass kernel for nn_Cell_A (capsule cell: conv1d -> squash ->
strided conv2d -> 3-iter dynamic routing).

Sharding: pure data parallel, batch B=8 across 8 NeuronCores. Each core runs
the same NEFF on its own batch element; weights replicated.

Per-core layout: l-major (sequence position on partitions, 8 chunks of 128).
 - conv1 in bf16 (PE), squash squares on ACT directly from PSUM.
 - conv2 runs 4-way row-tiled: quads (same r=cp%4, row blocks 0/32/64/96)
   execute concurrently in distinct 32x32 PE row groups.
 - routing multiplies on DVE bf16 (2x mode); reductions are PSUM-accumulated
   identity matmuls on the PE; the r2 logit update accumulates onto the r1
   PSUM bank (b2 = b1 + P(v1)) so no logit copy/add is needed.
 - transcendentals on ACT using only the exp/ln table set; 1/z = exp(-ln z);
   squash factor is sq*exp(-(ln(1+sq)+0.5*ln(sq+eps))).
 - small elementwise helpers (softmax c-scale, squash glue) on GPSIMD.
"""

import os
import sys

import numpy as np

sys.path.insert(0, "/opt/trn_rl_repo")

K, L = 64, 1024
CP, APd, G2 = 32, 8, 9
CSA, ASA, G3 = 16, 16, 3
NCH = CP * APd          # 256 conv1 out channels
NO = CSA * ASA          # 256 conv2 out channels
EPS = 1e-8
NCORES = 8
NCHUNK = 8              # L / 128
PL = 128                # l per chunk

_CACHE = {}
_B2_IS_ZERO = [True]


def _build_nc():
    import concourse.bacc as bacc
    import concourse.mybir as mybir
    import concourse.tile as tile
    from concourse.mybir import ActivationFunctionType as AF, AluOpType as OP

    f32 = mybir.dt.float32
    bf16 = mybir.dt.bfloat16

    # Pin all ACT activations to the one table set containing Exp+Ln+Copy so
    # the table-load pass emits a single hoisted load (no per-call reloads).
    from concourse.hw_specs import get_activation_tables as _gat
    _keep = "natural_log_exp_and_others"
    _used = {AF.Exp, AF.Ln, AF.Copy, AF.Identity, AF.Square}

    def _gat_one(arch):
        tabs = _gat(arch)
        assert _used <= tabs[_keep]
        return {n: (f if n == _keep else (f - _used)) for n, f in tabs.items()}

    bacc.get_activation_tables = _gat_one

    nc = bacc.Bacc("TRN2", target_bir_lowering=False, debug=False)

    x_d = nc.dram_tensor("x", [K, L], bf16, kind="ExternalInput")
    w1r_d = nc.dram_tensor("w1r", [K + 1, G2, NCH], bf16, kind="ExternalInput")
    w2bd_d = nc.dram_tensor("w2bd", [128, 4 * NO], bf16, kind="ExternalInput")
    w2s0_d = nc.dram_tensor("w2s0", [128, NO], bf16, kind="ExternalInput")
    b2f_d = nc.dram_tensor("b2f", [NO], f32, kind="ExternalInput")  # 2*b2 perm
    b2p_d = nc.dram_tensor("b2p", [NO], f32, kind="ExternalInput")  # b2 perm
    idbf_d = nc.dram_tensor("idbf", [128, 128], bf16, kind="ExternalInput")
    out_d = nc.dram_tensor("out", [L * CSA, ASA], f32, kind="ExternalOutput")
    DBG = bool(int(os.environ.get("KERNEL_DEBUG_V", "0")))
    if DBG:
        vdbg_d = nc.dram_tensor("vdbg", [NCHUNK, 128, CP * NO], bf16,
                                kind="ExternalOutput")
        ydbg_d = nc.dram_tensor("ydbg", [128, 8, L], bf16,
                                kind="ExternalOutput")
        sdbg_d = nc.dram_tensor("sdbg", [NCHUNK, 128, NO], f32,
                                kind="ExternalOutput")

    out_v = out_d.ap().rearrange("(l s) a -> l s a", s=CSA)

    with tile.TileContext(nc) as tc:
        import contextlib
        ctx = contextlib.ExitStack()
        with ctx:
            singles = ctx.enter_context(tc.tile_pool(name="singles", bufs=1))
            ysqt_p = ctx.enter_context(tc.tile_pool(name="ysqt", bufs=1))
            vpool = ctx.enter_context(tc.tile_pool(name="vpool", bufs=3))
            mpool = ctx.enter_context(tc.tile_pool(name="mpool", bufs=3))
            sm = ctx.enter_context(tc.tile_pool(name="sm", bufs=4))
            smb = ctx.enter_context(tc.tile_pool(name="smb", bufs=4))
            vout = ctx.enter_context(tc.tile_pool(name="vout", bufs=4))
            ps_a = ctx.enter_context(tc.tile_pool(name="ps_a", bufs=1, space="PSUM"))
            ps_s0 = ctx.enter_context(tc.tile_pool(name="ps_s0", bufs=1, space="PSUM"))
            ps_s = ctx.enter_context(tc.tile_pool(name="ps_s", bufs=2, space="PSUM"))
            ps_v = ctx.enter_context(tc.tile_pool(name="ps_v", bufs=2, space="PSUM"))
            ps_p = ctx.enter_context(tc.tile_pool(name="ps_p", bufs=2, space="PSUM"))

            # ---- constant / weight loads ----
            xpad = singles.tile([128, L + 8], bf16)
            nc.vector.memset(xpad[0:K, 0:4], 0.0)
            nc.vector.memset(xpad[0:K, L + 4:L + 8], 0.0)
            nc.vector.memset(xpad[K:K + 1, :], 1.0)
            nc.sync.dma_start(out=xpad[0:K, 4:4 + L], in_=x_d.ap())

            w1r = singles.tile([128, G2, NCH], bf16)
            nc.sync.dma_start(out=w1r[0:K + 1], in_=w1r_d.ap())
            w2bd = singles.tile([128, 4 * NO], bf16)
            nc.sync.dma_start(out=w2bd, in_=w2bd_d.ap())
            w2s0 = singles.tile([128, NO], bf16)
            nc.sync.dma_start(out=w2s0, in_=w2s0_d.ap())
            idbf = singles.tile([128, 128], bf16)
            nc.sync.dma_start(out=idbf, in_=idbf_d.ap())
            b2frep = singles.tile([128, NO], f32)
            nc.sync.dma_start(
                out=b2frep, in_=b2f_d.ap().unsqueeze(0).broadcast_to([128, NO]))
            b2prep = singles.tile([128, NO], f32)
            nc.sync.dma_start(
                out=b2prep, in_=b2p_d.ap().unsqueeze(0).broadcast_to([128, NO]))
            cst0 = singles.tile([128, 1], f32)
            nc.vector.memset(cst0, 0.0)
            cst1 = singles.tile([128, 1], f32)
            nc.vector.memset(cst1, 1.0)
            cstE = singles.tile([128, 1], f32)
            nc.vector.memset(cstE, EPS)

            # y_sqT: squashed conv1 output, channel-major (DMA staging)
            ysqt = [ysqt_p.tile([128, L], bf16, tag=f"ysqt{h}",
                                name=f"ysqt{h}") for h in range(2)]
            # yrep: per slot t (cps 4t..4t+3), row 32*dh + 8*g + ap holds
            # ysq[(4t+g, ap), l + dh - 1]; rows 96..127 are zero pad.
            yrep = ysqt_p.tile([128, 8, L], bf16, tag="yrep", name="yrep")
            # zero everything once: pad rows and the dh-shift edge columns
            # stay zero; the per-chunk shifted-copy DMAs fill the data rows.
            nc.gpsimd.memset(yrep, 0.0)

            def squash_factor(sq, pool, n, tg):
                """f = sq/((1+sq)*sqrt(sq+eps)) = sq*exp(-(ln(1+sq)+.5ln(sq+eps)))
                sq: [128, n] fp32. Returns f [128, n]. ACT + GPSIMD only."""
                l1 = pool.tile([128, n], f32, tag=f"{tg}_l1", name="sqf_l1")
                nc.scalar.activation(out=l1, in_=sq, func=AF.Ln,
                                     bias=cst1[:, 0:1], scale=1.0)
                l2 = pool.tile([128, n], f32, tag=f"{tg}_l2", name="sqf_l2")
                nc.scalar.activation(out=l2, in_=sq, func=AF.Ln,
                                     bias=cstE[:, 0:1], scale=1.0)
                t = pool.tile([128, n], f32, tag=f"{tg}_t", name="sqf_t")
                nc.vector.scalar_tensor_tensor(
                    out=t, in0=l2, scalar=0.5, in1=l1, op0=OP.mult, op1=OP.add)
                e = pool.tile([128, n], f32, tag=f"{tg}_e", name="sqf_e")
                nc.scalar.activation(out=e, in_=t, func=AF.Exp,
                                     bias=cst0[:, 0:1], scale=-1.0)
                f = pool.tile([128, n], f32, tag=f"{tg}_f", name="sqf_f")
                nc.vector.tensor_tensor(out=f, in0=sq, in1=e, op=OP.mult)
                return f

            # -------------- stage A: conv1 + squash + transpose --------------
            def stage_a(c):
                yps = ps_a.tile([128, NCH], f32, tag="aps", name="yps")
                for t in range(G2):
                    nc.tensor.matmul(
                        yps, lhsT=xpad[0:K + 1, c * PL + t: c * PL + t + PL],
                        rhs=w1r[0:K + 1, t, :],
                        start=(t == 0), stop=(t == G2 - 1))
                y2 = sm.tile([128, NCH], f32, tag="y2", name="y2")
                nc.scalar.activation(out=y2, in_=yps, func=AF.Square,
                                     bias=cst0[:, 0:1], scale=1.0)
                sq = sm.tile([128, CP], f32, tag="sq1", name="sq1")
                nc.vector.tensor_reduce(
                    out=sq, in_=y2.rearrange("p (c a) -> p c a", a=APd),
                    axis=mybir.AxisListType.X, op=OP.add)
                f = squash_factor(sq, sm, CP, "sa")
                ysq = sm.tile([128, NCH], bf16, tag="ysq", name="ysq")
                nc.vector.tensor_tensor(
                    out=ysq.rearrange("p (c a) -> p c a", a=APd),
                    in0=yps.rearrange("p (c a) -> p c a", a=APd),
                    in1=f.unsqueeze(2).broadcast_to([128, CP, APd]),
                    op=OP.mult)
                for h in range(2):
                    tps = ps_a.tile([128, 128], bf16, tag="aps", name="tps")
                    nc.tensor.transpose(
                        tps, in_=ysq[:, h * 128:(h + 1) * 128], identity=idbf)
                    nc.scalar.copy(
                        out=ysqt[h][:, c * PL: (c + 1) * PL], in_=tps)
                # replicate into yrep: 3 dh-shifted contiguous 32-row copies
                # per slot, on the otherwise-idle GPSIMD engine.
                for dh in range(G3):
                    sh = 1 - dh
                    s_lo = c * PL
                    s_hi = c * PL + PL
                    d_lo, d_hi = s_lo + sh, s_hi + sh
                    if d_lo < 0:
                        s_lo += -d_lo
                        d_lo = 0
                    if d_hi > L:
                        s_hi -= d_hi - L
                        d_hi = L
                    for t in range(8):
                        h = t // 4
                        tl = t - 4 * h
                        nc.gpsimd.tensor_copy(
                            yrep[32 * dh:32 * dh + 32, t, d_lo:d_hi],
                            ysqt[h][32 * tl:32 * tl + 32, s_lo:s_hi])

            # ---------- stage B: conv2 (+s0) per chunk ----------
            vt_of = {}
            s0_of = {}

            def conv2_s0(c):
                # s0 = (1/16) sum_cp V: yrep slots vs w2s0, PSUM-accumulated
                s0ps = ps_s0.tile([128, NO], f32, tag="s0ps", name="s0ps")
                for t in range(8):
                    nc.tensor.matmul(
                        s0ps, lhsT=yrep[:, t, c * PL:(c + 1) * PL],
                        rhs=w2s0, start=(t == 0), stop=(t == 7))

                # conv2 -> V chunk, bf16, layout [128, cp, a, csa]
                # slot t covers cps 4t..4t+3; block-diagonal w2bd gives two
                # cps per plain 512-col matmul (out cols (g, (a,csa))).
                vt = vpool.tile([128, CP, ASA, CSA], bf16, tag="vt", name="vt")
                for t in range(8):
                    for j in range(2):
                        vq = ps_v.tile([128, 2 * NO], f32, tag="vq", name="vq")
                        nc.tensor.matmul(
                            vq, lhsT=yrep[:, t, c * PL:(c + 1) * PL],
                            rhs=w2bd[:, 2 * j * NO:(2 * j + 2) * NO],
                            start=True, stop=True)
                        dst = vt[:, 4 * t + 2 * j:4 * t + 2 * j + 2, :, :]
                        if _B2_IS_ZERO[0]:
                            nc.scalar.copy(out=dst, in_=vq)
                        else:
                            nc.vector.scalar_tensor_tensor(
                                out=dst.rearrange("p c a s -> p (c a s)"),
                                in0=vq, scalar=1.0,
                                in1=b2prep.unsqueeze(1).broadcast_to(
                                    [128, 2, NO]).rearrange(
                                    "p c s -> p (c s)"),
                                op0=OP.mult, op1=OP.add)

                if DBG:
                    nc.sync.dma_start(
                        out=vdbg_d.ap()[c],
                        in_=vt.rearrange("p c a s -> p (c a s)"))
                    sdbgs = smb.tile([128, NO], f32, tag="sdbg", name="sdbgs")
                    nc.scalar.copy(out=sdbgs, in_=s0ps)
                    nc.sync.dma_start(out=sdbg_d.ap()[c], in_=sdbgs)
                    if c == NCHUNK - 1:
                        nc.sync.dma_start(out=ydbg_d.ap(), in_=yrep)
                vt_of[c] = vt
                s0_of[c] = s0ps

            # ---------- stage C: routing per chunk ----------
            def squash_psum(spsrc, tg, out_tile, out_view):
                """squash(s) from PSUM s [128, (a s)]; writes out_tile."""
                s2 = smb.tile([128, NO], f32, tag=f"{tg}_s2", name="s2")
                nc.scalar.activation(out=s2, in_=spsrc, func=AF.Square,
                                     bias=cst0[:, 0:1], scale=1.0)
                sqs = smb.tile([128, CSA], f32, tag=f"{tg}_sqs", name="sqs")
                nc.vector.tensor_reduce(
                    out=sqs,
                    in_=s2.rearrange("p (a s) -> p a s", s=CSA)
                    .transpose([0, 2, 1]),
                    axis=mybir.AxisListType.X, op=OP.add)
                fs = squash_factor(sqs, smb, CSA, tg)
                nc.vector.tensor_tensor(
                    out=out_view,
                    in0=spsrc.rearrange("p (a s) -> p a s", s=CSA),
                    in1=fs.unsqueeze(1).broadcast_to([128, ASA, CSA]),
                    op=OP.mult)
                return out_tile

            def routing(c):
                vt = vt_of.pop(c)
                s0ps = s0_of.pop(c)
                if _B2_IS_ZERO[0]:
                    s_src = s0ps
                else:
                    s0b = smb.tile([128, NO], f32, tag="s0b", name="s0b")
                    nc.vector.tensor_add(s0b, s0ps, b2frep)
                    s_src = s0b

                pps = ps_p.tile([128, CP * CSA], f32, tag="pps", name="pps")
                sps_last = None
                for r in (1, 2):
                    vprev = vout.tile([128, ASA, CSA], bf16, tag="vprev",
                                      name="vprev")
                    squash_psum(s_src, f"r{r}", vprev, vprev)

                    # P-op: M = V * v_prev (bcast over cp); P = sum_a M
                    mt = mpool.tile([128, CP, ASA, CSA], bf16, tag="mt",
                                    name="mt")
                    nc.vector.tensor_tensor(
                        out=mt, in0=vt,
                        in1=vprev.unsqueeze(1).broadcast_to(
                            [128, CP, ASA, CSA]),
                        op=OP.mult)
                    # logits accumulate in PSUM across iterations (b = sum P)
                    for ai in range(ASA):
                        nc.tensor.matmul(
                            pps.rearrange("p (c s) -> p c s", s=CSA),
                            lhsT=idbf, rhs=mt[:, :, ai, :],
                            start=(r == 1 and ai == 0), stop=(ai == ASA - 1),
                            skip_group_check=True)
                    # softmax over csa (no max-sub; logits are small)
                    et = smb.tile([128, CP, CSA], bf16, tag="et", name="et")
                    nc.scalar.activation(
                        out=et, in_=pps.rearrange("p (c s) -> p c s", s=CSA),
                        func=AF.Exp, bias=cst0[:, 0:1], scale=1.0)
                    zt = smb.tile([128, CP], f32, tag="zt", name="zt")
                    nc.vector.tensor_reduce(
                        out=zt, in_=et, axis=mybir.AxisListType.X, op=OP.add)
                    lnz = smb.tile([128, CP], f32, tag="lnz", name="lnz")
                    nc.scalar.activation(out=lnz, in_=zt, func=AF.Ln,
                                         bias=cst0[:, 0:1], scale=1.0)
                    rz = smb.tile([128, CP], f32, tag="rz", name="rz")
                    nc.scalar.activation(out=rz, in_=lnz, func=AF.Exp,
                                         bias=cst0[:, 0:1], scale=-1.0)
                    ct = smb.tile([128, CP, CSA], bf16, tag="ct", name="ct")
                    nc.vector.tensor_tensor(
                        out=ct, in0=et,
                        in1=rz.unsqueeze(2).broadcast_to([128, CP, CSA]),
                        op=OP.mult)

                    # s-op: M2 = V * c (bcast over a); s = sum_cp M2
                    mt2 = mpool.tile([128, CP, ASA, CSA], bf16, tag="mt",
                                     name="mt2")
                    nc.vector.tensor_tensor(
                        out=mt2, in0=vt,
                        in1=ct.unsqueeze(2).broadcast_to([128, CP, ASA, CSA]),
                        op=OP.mult)
                    sps = ps_s.tile([128, NO], f32, tag="sps", name="sps")
                    for cpi in range(CP):
                        nc.tensor.matmul(
                            sps, lhsT=idbf,
                            rhs=mt2[:, cpi, :, :].rearrange("p a s -> p (a s)"),
                            start=(cpi == 0), stop=(cpi == CP - 1))
                    s_src = sps
                    sps_last = sps

                # final squash of last s -> v2, DMA out
                v2 = vout.tile([128, CSA, ASA], f32, tag="v2", name="v2")
                squash_psum(sps_last, "rf", v2, v2.transpose([0, 2, 1]))
                nc.sync.dma_start(
                    out=out_v[c * PL:(c + 1) * PL], in_=v2)

            # ---------- 3-stage software pipeline ----------
            for c in range(NCHUNK + 2):
                if c < NCHUNK:
                    stage_a(c)
                if 0 <= c - 1 < NCHUNK:
                    conv2_s0(c - 1)
                if 0 <= c - 2 < NCHUNK:
                    routing(c - 2)
    nc.compile()
    return nc


def _prep_weights(w1, b1, w2, b2):
    import ml_dtypes
    w1 = np.asarray(w1, np.float32)
    w2 = np.asarray(w2, np.float32)
    b1 = np.asarray(b1, np.float32)
    b2 = np.asarray(b2, np.float32)
    # o-permutation: new column order (a, csa): perm[a*CSA+csa] = csa*ASA+a
    a_i, s_i = np.meshgrid(np.arange(ASA), np.arange(CSA), indexing="ij")
    perm = (s_i * ASA + a_i).reshape(-1)
    w1r = np.zeros((K + 1, G2, NCH), np.float32)
    w1r[0:K] = np.transpose(w1, (1, 2, 0))          # [k, t, o]
    w1r[K, (G2 - 1) // 2, :] = b1                    # bias via ones-row
    w2m = w2[:, 0, :, :]                             # [o, dh, ap]
    w2p = w2m[perm]                                  # permuted o
    # rows 32*dh + 8*g + ap; cols (g', o') block-diagonal
    w2bd = np.zeros((128, 4 * NO), np.float32)
    w2s0 = np.zeros((128, NO), np.float32)
    for g in range(4):
        for dh in range(G3):
            rows = slice(32 * dh + 8 * g, 32 * dh + 8 * g + 8)
            w2bd[rows, g * NO:(g + 1) * NO] = w2p[:, dh, :].T
            w2s0[rows, :] = w2p[:, dh, :].T / float(CSA)
    return {
        "w1r": w1r.astype(ml_dtypes.bfloat16),
        "w2bd": w2bd.astype(ml_dtypes.bfloat16),
        "w2s0": w2s0.astype(ml_dtypes.bfloat16),
        "b2p": np.ascontiguousarray(b2[perm]),
        "b2f": np.ascontiguousarray(2.0 * b2[perm]).astype(np.float32),
        "idbf": np.eye(128).astype(ml_dtypes.bfloat16),
    }


def kernel(x, w1, b1, w2, b2):
    import ml_dtypes
    from concourse.bass_utils import run_bass_kernel_spmd

    x = np.asarray(x, np.float32)
    _B2_IS_ZERO[0] = bool(np.all(np.asarray(b2) == 0.0))
    if "nc" not in _CACHE:
        _CACHE["nc"] = _build_nc()
    nc = _CACHE["nc"]
    wmaps = _prep_weights(w1, b1, w2, b2)
    in_maps = []
    for b in range(NCORES):
        m = {"x": np.ascontiguousarray(x[b]).astype(ml_dtypes.bfloat16)}
        m.update(wmaps)
        in_maps.append(m)
    trace = bool(int(os.environ.get("KERNEL_TRACE", "0")))
    res = run_bass_kernel_spmd(
        nc, in_maps, core_ids=list(range(NCORES)), trace=trace)
    if trace:
        _CACHE["last_exec_time_ns"] = res.exec_time_ns
        _CACHE["last_trace"] = res.instructions_and_trace
    out = np.stack([r["out"] for r in res.results])   # [8, 16384, 16]
    return out.astype(np.float32)


# revision 17
# speedup vs baseline: 1.3323x; 1.0955x over previous
"""Trainium2 Bass kernel for nn_Cell_A (capsule cell: conv1d -> squash ->
strided conv2d -> 3-iter dynamic routing).

Sharding: pure data parallel, batch B=8 across 8 NeuronCores. Each core runs
the same NEFF on its own batch element; weights replicated.

Per-core layout: l-major (sequence position on partitions, 8 chunks of 128).
 - conv1 in bf16 (PE), squash squares on ACT directly from PSUM.
 - conv2 runs 4-way row-tiled: quads (same r=cp%4, row blocks 0/32/64/96)
   execute concurrently in distinct 32x32 PE row groups.
 - routing multiplies on DVE bf16 (2x mode); reductions are PSUM-accumulated
   identity matmuls on the PE; the r2 logit update accumulates onto the r1
   PSUM bank (b2 = b1 + P(v1)) so no logit copy/add is needed.
 - transcendentals on ACT using only the exp/ln table set; 1/z = exp(-ln z);
   squash factor is sq*exp(-(ln(1+sq)+0.5*ln(sq+eps))).
 - small elementwise helpers (softmax c-scale, squash glue) on GPSIMD.
"""

import os
import sys

import numpy as np

sys.path.insert(0, "/opt/trn_rl_repo")

K, L = 64, 1024
CP, APd, G2 = 32, 8, 9
CSA, ASA, G3 = 16, 16, 3
NCH = CP * APd          # 256 conv1 out channels
NO = CSA * ASA          # 256 conv2 out channels
EPS = 1e-8
NCORES = 8
NCHUNK = 8              # L / 128
PL = 128                # l per chunk

_CACHE = {}
_B2_IS_ZERO = [True]


def _build_nc():
    import concourse.bacc as bacc
    import concourse.mybir as mybir
    import concourse.tile as tile
    from concourse.mybir import ActivationFunctionType as AF, AluOpType as OP

    f32 = mybir.dt.float32
    bf16 = mybir.dt.bfloat16

    # Pin all ACT activations to the one table set containing Exp+Ln+Copy so
    # the table-load pass emits a single hoisted load (no per-call reloads).
    from concourse.hw_specs import get_activation_tables as _gat
    _keep = "natural_log_exp_and_others"
    _used = {AF.Exp, AF.Ln, AF.Copy, AF.Identity, AF.Square}

    def _gat_one(arch):
        tabs = _gat(arch)
        assert _used <= tabs[_keep]
        return {n: (f if n == _keep else (f - _used)) for n, f in tabs.items()}

    bacc.get_activation_tables = _gat_one

    nc = bacc.Bacc("TRN2", target_bir_lowering=False, debug=False)

    x_d = nc.dram_tensor("x", [K, L], bf16, kind="ExternalInput")
    w1r_d = nc.dram_tensor("w1r", [K + 1, G2, NCH], bf16, kind="ExternalInput")
    w2bd_d = nc.dram_tensor("w2bd", [128, 4 * NO], bf16, kind="ExternalInput")
    w2s0_d = nc.dram_tensor("w2s0", [128, NO], bf16, kind="ExternalInput")
    b2f_d = nc.dram_tensor("b2f", [NO], f32, kind="ExternalInput")  # 2*b2 perm
    b2p_d = nc.dram_tensor("b2p", [NO], f32, kind="ExternalInput")  # b2 perm
    idbf_d = nc.dram_tensor("idbf", [128, 128], bf16, kind="ExternalInput")
    out_d = nc.dram_tensor("out", [L * CSA, ASA], f32, kind="ExternalOutput")
    DBG = bool(int(os.environ.get("KERNEL_DEBUG_V", "0")))
    if DBG:
        vdbg_d = nc.dram_tensor("vdbg", [NCHUNK, 128, CP * NO], bf16,
                                kind="ExternalOutput")
        ydbg_d = nc.dram_tensor("ydbg", [128, 8, L], bf16,
                                kind="ExternalOutput")
        sdbg_d = nc.dram_tensor("sdbg", [NCHUNK, 128, NO], f32,
                                kind="ExternalOutput")

    out_v = out_d.ap().rearrange("(l s) a -> l s a", s=CSA)

    with tile.TileContext(nc) as tc:
        import contextlib
        ctx = contextlib.ExitStack()
        with ctx:
            singles = ctx.enter_context(tc.tile_pool(name="singles", bufs=1))
            ysqt_p = ctx.enter_context(tc.tile_pool(name="ysqt", bufs=1))
            vpool = ctx.enter_context(tc.tile_pool(name="vpool", bufs=3))
            mpool = ctx.enter_context(tc.tile_pool(name="mpool", bufs=3))
            sm = ctx.enter_context(tc.tile_pool(name="sm", bufs=4))
            smb = ctx.enter_context(tc.tile_pool(name="smb", bufs=4))
            vout = ctx.enter_context(tc.tile_pool(name="vout", bufs=4))
            ps_a = ctx.enter_context(tc.tile_pool(name="ps_a", bufs=1, space="PSUM"))
            ps_s0 = ctx.enter_context(tc.tile_pool(name="ps_s0", bufs=1, space="PSUM"))
            ps_s = ctx.enter_context(tc.tile_pool(name="ps_s", bufs=2, space="PSUM"))
            ps_v = ctx.enter_context(tc.tile_pool(name="ps_v", bufs=2, space="PSUM"))
            ps_p = ctx.enter_context(tc.tile_pool(name="ps_p", bufs=2, space="PSUM"))

            # ---- constant / weight loads ----
            xpad = singles.tile([128, L + 8], bf16)
            nc.vector.memset(xpad[0:K, 0:4], 0.0)
            nc.vector.memset(xpad[0:K, L + 4:L + 8], 0.0)
            nc.vector.memset(xpad[K:K + 1, :], 1.0)
            nc.sync.dma_start(out=xpad[0:K, 4:4 + L], in_=x_d.ap())

            w1r = singles.tile([128, G2, NCH], bf16)
            nc.sync.dma_start(out=w1r[0:K + 1], in_=w1r_d.ap())
            w2bd = singles.tile([128, 4 * NO], bf16)
            nc.sync.dma_start(out=w2bd, in_=w2bd_d.ap())
            w2s0 = singles.tile([128, NO], bf16)
            nc.sync.dma_start(out=w2s0, in_=w2s0_d.ap())
            idbf = singles.tile([128, 128], bf16)
            nc.sync.dma_start(out=idbf, in_=idbf_d.ap())
            b2frep = singles.tile([128, NO], f32)
            nc.sync.dma_start(
                out=b2frep, in_=b2f_d.ap().unsqueeze(0).broadcast_to([128, NO]))
            b2prep = singles.tile([128, NO], f32)
            nc.sync.dma_start(
                out=b2prep, in_=b2p_d.ap().unsqueeze(0).broadcast_to([128, NO]))
            cst0 = singles.tile([128, 1], f32)
            nc.vector.memset(cst0, 0.0)
            cst1 = singles.tile([128, 1], f32)
            nc.vector.memset(cst1, 1.0)
            cstE = singles.tile([128, 1], f32)
            nc.vector.memset(cstE, EPS)

            # y_sqT: squashed conv1 output, channel-major (DMA staging)
            ysqt = [ysqt_p.tile([128, L], bf16, tag=f"ysqt{h}",
                                name=f"ysqt{h}") for h in range(2)]
            # yrep: per slot t (cps 4t..4t+3), row 32*dh + 8*g + ap holds
            # ysq[(4t+g, ap), l + dh - 1]; rows 96..127 are zero pad.
            yrep = ysqt_p.tile([128, 8, L], bf16, tag="yrep", name="yrep")
            # zero everything once: pad rows and the dh-shift edge columns
            # stay zero; the per-chunk shifted-copy DMAs fill the data rows.
            nc.gpsimd.memset(yrep, 0.0)

            def squash_factor(sq, pool, n, tg):
                """f = sq/((1+sq)*sqrt(sq+eps)) = sq*exp(-(ln(1+sq)+.5ln(sq+eps)))
                sq: [128, n] fp32. Returns f [128, n]. ACT + GPSIMD only."""
                l1 = pool.tile([128, n], f32, tag=f"{tg}_l1", name="sqf_l1")
                nc.scalar.activation(out=l1, in_=sq, func=AF.Ln,
                                     bias=cst1[:, 0:1], scale=1.0)
                l2 = pool.tile([128, n], f32, tag=f"{tg}_l2", name="sqf_l2")
                nc.scalar.activation(out=l2, in_=sq, func=AF.Ln,
                                     bias=cstE[:, 0:1], scale=1.0)
                t = pool.tile([128, n], f32, tag=f"{tg}_t", name="sqf_t")
                nc.vector.scalar_tensor_tensor(
                    out=t, in0=l2, scalar=0.5, in1=l1, op0=OP.mult, op1=OP.add)
                e = pool.tile([128, n], f32, tag=f"{tg}_e", name="sqf_e")
                nc.scalar.activation(out=e, in_=t, func=AF.Exp,
                                     bias=cst0[:, 0:1], scale=-1.0)
                f = pool.tile([128, n], f32, tag=f"{tg}_f", name="sqf_f")
                nc.vector.tensor_tensor(out=f, in0=sq, in1=e, op=OP.mult)
                return f

            # -------------- stage A: conv1 + squash + transpose --------------
            def stage_a(c):
                yps = ps_a.tile([128, NCH], f32, tag="aps", name="yps")
                for t in range(G2):
                    nc.tensor.matmul(
                        yps, lhsT=xpad[0:K + 1, c * PL + t: c * PL + t + PL],
                        rhs=w1r[0:K + 1, t, :],
                        start=(t == 0), stop=(t == G2 - 1))
                y2 = sm.tile([128, NCH], f32, tag="y2", name="y2")
                nc.scalar.activation(out=y2, in_=yps, func=AF.Square,
                                     bias=cst0[:, 0:1], scale=1.0)
                sq = sm.tile([128, CP], f32, tag="sq1", name="sq1")
                nc.vector.tensor_reduce(
                    out=sq, in_=y2.rearrange("p (c a) -> p c a", a=APd),
                    axis=mybir.AxisListType.X, op=OP.add)
                f = squash_factor(sq, sm, CP, "sa")
                ysq = sm.tile([128, NCH], bf16, tag="ysq", name="ysq")
                nc.vector.tensor_tensor(
                    out=ysq.rearrange("p (c a) -> p c a", a=APd),
                    in0=yps.rearrange("p (c a) -> p c a", a=APd),
                    in1=f.unsqueeze(2).broadcast_to([128, CP, APd]),
                    op=OP.mult)
                for h in range(2):
                    tps = ps_a.tile([128, 128], bf16, tag="aps", name="tps")
                    nc.tensor.transpose(
                        tps, in_=ysq[:, h * 128:(h + 1) * 128], identity=idbf)
                    nc.scalar.copy(
                        out=ysqt[h][:, c * PL: (c + 1) * PL], in_=tps)
                # replicate into yrep: 3 dh-shifted contiguous 32-row copies
                # per slot; cheap on DVE (bf16 sbuf->sbuf 2x/4x mode).
                for dh in range(G3):
                    sh = 1 - dh
                    s_lo = c * PL
                    s_hi = c * PL + PL
                    d_lo, d_hi = s_lo + sh, s_hi + sh
                    if d_lo < 0:
                        s_lo += -d_lo
                        d_lo = 0
                    if d_hi > L:
                        s_hi -= d_hi - L
                        d_hi = L
                    for t in range(8):
                        h = t // 4
                        tl = t - 4 * h
                        nc.vector.tensor_copy(
                            yrep[32 * dh:32 * dh + 32, t, d_lo:d_hi],
                            ysqt[h][32 * tl:32 * tl + 32, s_lo:s_hi])

            # ---------- stage B: conv2 (+s0) per chunk ----------
            vt_of = {}
            s0_of = {}

            def conv2_s0(c):
                # s0 = (1/16) sum_cp V: yrep slots vs w2s0, PSUM-accumulated
                s0ps = ps_s0.tile([128, NO], f32, tag="s0ps", name="s0ps")
                for t in range(8):
                    nc.tensor.matmul(
                        s0ps, lhsT=yrep[:, t, c * PL:(c + 1) * PL],
                        rhs=w2s0, start=(t == 0), stop=(t == 7))

                # conv2 -> V chunk, bf16, layout [128, cp, a, csa]
                # slot t covers cps 4t..4t+3; block-diagonal w2bd gives two
                # cps per plain 512-col matmul (out cols (g, (a,csa))).
                vt = vpool.tile([128, CP, ASA, CSA], bf16, tag="vt", name="vt")
                for t in range(8):
                    for j in range(2):
                        vq = ps_v.tile([128, 2 * NO], f32, tag="vq", name="vq")
                        nc.tensor.matmul(
                            vq, lhsT=yrep[:, t, c * PL:(c + 1) * PL],
                            rhs=w2bd[:, 2 * j * NO:(2 * j + 2) * NO],
                            start=True, stop=True)
                        dst = vt[:, 4 * t + 2 * j:4 * t + 2 * j + 2, :, :]
                        if _B2_IS_ZERO[0]:
                            nc.scalar.copy(out=dst, in_=vq)
                        else:
                            nc.vector.scalar_tensor_tensor(
                                out=dst.rearrange("p c a s -> p (c a s)"),
                                in0=vq, scalar=1.0,
                                in1=b2prep.unsqueeze(1).broadcast_to(
                                    [128, 2, NO]).rearrange(
                                    "p c s -> p (c s)"),
                                op0=OP.mult, op1=OP.add)

                if DBG:
                    nc.sync.dma_start(
                        out=vdbg_d.ap()[c],
                        in_=vt.rearrange("p c a s -> p (c a s)"))
                    sdbgs = smb.tile([128, NO], f32, tag="sdbg", name="sdbgs")
                    nc.scalar.copy(out=sdbgs, in_=s0ps)
                    nc.sync.dma_start(out=sdbg_d.ap()[c], in_=sdbgs)
                    if c == NCHUNK - 1:
                        nc.sync.dma_start(out=ydbg_d.ap(), in_=yrep)
                vt_of[c] = vt
                s0_of[c] = s0ps

            # ---------- stage C: routing per chunk ----------
            def squash_psum(spsrc, tg, out_tile, out_view):
                """squash(s) from PSUM s [128, (a s)]; writes out_tile."""
                s2 = smb.tile([128, NO], f32, tag=f"{tg}_s2", name="s2")
                nc.scalar.activation(out=s2, in_=spsrc, func=AF.Square,
                                     bias=cst0[:, 0:1], scale=1.0)
                sqs = smb.tile([128, CSA], f32, tag=f"{tg}_sqs", name="sqs")
                nc.vector.tensor_reduce(
                    out=sqs,
                    in_=s2.rearrange("p (a s) -> p a s", s=CSA)
                    .transpose([0, 2, 1]),
                    axis=mybir.AxisListType.X, op=OP.add)
                fs = squash_factor(sqs, smb, CSA, tg)
                nc.vector.tensor_tensor(
                    out=out_view,
                    in0=spsrc.rearrange("p (a s) -> p a s", s=CSA),
                    in1=fs.unsqueeze(1).broadcast_to([128, ASA, CSA]),
                    op=OP.mult)
                return out_tile

            def routing(c):
                vt = vt_of.pop(c)
                s0ps = s0_of.pop(c)
                if _B2_IS_ZERO[0]:
                    s_src = s0ps
                else:
                    s0b = smb.tile([128, NO], f32, tag="s0b", name="s0b")
                    nc.vector.tensor_add(s0b, s0ps, b2frep)
                    s_src = s0b

                pps = ps_p.tile([128, CP * CSA], f32, tag="pps", name="pps")
                sps_last = None
                for r in (1, 2):
                    vprev = vout.tile([128, ASA, CSA], bf16, tag="vprev",
                                      name="vprev")
                    squash_psum(s_src, f"r{r}", vprev, vprev)

                    # P-op: M = V * v_prev (bcast over cp); P = sum_a M
                    mt = mpool.tile([128, CP, ASA, CSA], bf16, tag="mt",
                                    name="mt")
                    nc.vector.tensor_tensor(
                        out=mt, in0=vt,
                        in1=vprev.unsqueeze(1).broadcast_to(
                            [128, CP, ASA, CSA]),
                        op=OP.mult)
                    # logits accumulate in PSUM across iterations (b = sum P)
                    for ai in range(ASA):
                        nc.tensor.matmul(
                            pps.rearrange("p (c s) -> p c s", s=CSA),
                            lhsT=idbf, rhs=mt[:, :, ai, :],
                            start=(r == 1 and ai == 0), stop=(ai == ASA - 1),
                            skip_group_check=True)
                    # softmax over csa (no max-sub; logits are small)
                    et = smb.tile([128, CP, CSA], bf16, tag="et", name="et")
                    nc.scalar.activation(
                        out=et, in_=pps.rearrange("p (c s) -> p c s", s=CSA),
                        func=AF.Exp, bias=cst0[:, 0:1], scale=1.0)
                    zt = smb.tile([128, CP], f32, tag="zt", name="zt")
                    nc.vector.tensor_reduce(
                        out=zt, in_=et, axis=mybir.AxisListType.X, op=OP.add)
                    lnz = smb.tile([128, CP], f32, tag="lnz", name="lnz")
                    nc.scalar.activation(out=lnz, in_=zt, func=AF.Ln,
                                         bias=cst0[:, 0:1], scale=1.0)
                    rz = smb.tile([128, CP], f32, tag="rz", name="rz")
                    nc.scalar.activation(out=rz, in_=lnz, func=AF.Exp,
                                         bias=cst0[:, 0:1], scale=-1.0)
                    ct = smb.tile([128, CP, CSA], bf16, tag="ct", name="ct")
                    nc.vector.tensor_tensor(
                        out=ct, in0=et,
                        in1=rz.unsqueeze(2).broadcast_to([128, CP, CSA]),
                        op=OP.mult)

                    # s-op: M2 = V * c (bcast over a); s = sum_cp M2
                    mt2 = mpool.tile([128, CP, ASA, CSA], bf16, tag="mt",
                                     name="mt2")
                    nc.vector.tensor_tensor(
                        out=mt2, in0=vt,
                        in1=ct.unsqueeze(2).broadcast_to([128, CP, ASA, CSA]),
                        op=OP.mult)
                    sps = ps_s.tile([128, NO], f32, tag="sps", name="sps")
                    for cpi in range(CP):
                        nc.tensor.matmul(
                            sps, lhsT=idbf,
                            rhs=mt2[:, cpi, :, :].rearrange("p a s -> p (a s)"),
                            start=(cpi == 0), stop=(cpi == CP - 1))
                    s_src = sps
                    sps_last = sps

                # final squash of last s -> v2, DMA out
                v2 = vout.tile([128, CSA, ASA], f32, tag="v2", name="v2")
                squash_psum(sps_last, "rf", v2, v2.transpose([0, 2, 1]))
                nc.sync.dma_start(
                    out=out_v[c * PL:(c + 1) * PL], in_=v2)

            # ---------- 3-stage software pipeline ----------
            for c in range(NCHUNK + 2):
                if c < NCHUNK:
                    stage_a(c)
                if 0 <= c - 1 < NCHUNK:
                    conv2_s0(c - 1)
                if 0 <= c - 2 < NCHUNK:
                    routing(c - 2)
    nc.compile()
    return nc


def _prep_weights(w1, b1, w2, b2):
    import ml_dtypes
    w1 = np.asarray(w1, np.float32)
    w2 = np.asarray(w2, np.float32)
    b1 = np.asarray(b1, np.float32)
    b2 = np.asarray(b2, np.float32)
    # o-permutation: new column order (a, csa): perm[a*CSA+csa] = csa*ASA+a
    a_i, s_i = np.meshgrid(np.arange(ASA), np.arange(CSA), indexing="ij")
    perm = (s_i * ASA + a_i).reshape(-1)
    w1r = np.zeros((K + 1, G2, NCH), np.float32)
    w1r[0:K] = np.transpose(w1, (1, 2, 0))          # [k, t, o]
    w1r[K, (G2 - 1) // 2, :] = b1                    # bias via ones-row
    w2m = w2[:, 0, :, :]                             # [o, dh, ap]
    w2p = w2m[perm]                                  # permuted o
    # rows 32*dh + 8*g + ap; cols (g', o') block-diagonal
    w2bd = np.zeros((128, 4 * NO), np.float32)
    w2s0 = np.zeros((128, NO), np.float32)
    for g in range(4):
        for dh in range(G3):
            rows = slice(32 * dh + 8 * g, 32 * dh + 8 * g + 8)
            w2bd[rows, g * NO:(g + 1) * NO] = w2p[:, dh, :].T
            w2s0[rows, :] = w2p[:, dh, :].T / float(CSA)
    return {
        "w1r": w1r.astype(ml_dtypes.bfloat16),
        "w2bd": w2bd.astype(ml_dtypes.bfloat16),
        "w2s0": w2s0.astype(ml_dtypes.bfloat16),
        "b2p": np.ascontiguousarray(b2[perm]),
        "b2f": np.ascontiguousarray(2.0 * b2[perm]).astype(np.float32),
        "idbf": np.eye(128).astype(ml_dtypes.bfloat16),
    }


def kernel(x, w1, b1, w2, b2):
    import ml_dtypes
    from concourse.bass_utils import run_bass_kernel_spmd

    x = np.asarray(x, np.float32)
    _B2_IS_ZERO[0] = bool(np.all(np.asarray(b2) == 0.0))
    if "nc" not in _CACHE:
        _CACHE["nc"] = _build_nc()
    nc = _CACHE["nc"]
    wmaps = _prep_weights(w1, b1, w2, b2)
    in_maps = []
    for b in range(NCORES):
        m = {"x": np.ascontiguousarray(x[b]).astype(ml_dtypes.bfloat16)}
        m.update(wmaps)
        in_maps.append(m)
    trace = bool(int(os.environ.get("KERNEL_TRACE", "0")))
    res = run_bass_kernel_spmd(
        nc, in_maps, core_ids=list(range(NCORES)), trace=trace)
    if trace:
        _CACHE["last_exec_time_ns"] = res.exec_time_ns
        _CACHE["last_trace"] = res.instructions_and_trace
    out = np.stack([r["out"] for r in res.results])   # [8, 16384, 16]
    return out.astype(np.float32)
